# revision 17
# baseline (speedup 1.0000x reference)
"""GatedGCN LocalEncoder kernel for 8x Trainium2 NeuronCores (Bass/Tile).

Strategy: destination-sorted edge sharding with dma_gather-based source
gathers. Nodes are relabeled into degree-balanced 128-node blocks (784
blocks, 98 per core). All edges with dst in a block form one contiguous
run, sub-sorted by source quarter (4 tables of 25088 rows so gather
indices fit int16). Per block:
  - 4 transposed dma_gathers fetch x[src]^T (bf16) from the quarter tables
  - 1 SBUF-source dma_gather fetches onehot(dst_local) columns from a
    resident identity tile (used to broadcast the per-block Vh and the u2
    bias into the gate PSUM with a single matmul)
  - per 128-edge chunk: 3 gate matmuls (A|U projection from gathered x^T,
    edge-feature term, Vh/u2 one-hot term), sigmoid, msg multiply, and a
    one-hot segment-sum matmul accumulated in PSUM
  - block tail: residual + LayerNorm as two f32 matmuls + DVE ops.
No cross-core communication; each core owns 98 blocks of destinations.
"""

import os
import sys
from contextlib import ExitStack

for _p in ("/opt/trn_rl_repo", os.path.expanduser("~/.axon_site/_ro/trn_rl_repo")):
    if os.path.isdir(_p) and _p not in sys.path:
        sys.path.insert(0, _p)

import numpy as np
import ml_dtypes

import concourse.bass as bass
import concourse.mybir as mybir
import concourse.tile as tile
from concourse import bacc
from concourse import bass_utils
from concourse import library_config

BF16 = mybir.dt.bfloat16
F32 = mybir.dt.float32
I16 = mybir.dt.int16
P = 128
NCORES = 8
NQ = 4  # source-quarter tables (int16 gather index limit)

LAST_RESULTS = None  # test harness introspection


def _wrap_idx16(vals, runs, run_len):
    """vals [runs*run_len] int -> [128, runs*run_len//16] int16 in dma_gather's
    wrapped layout (idx i of a run -> partition i%16, col i//16; replicated
    across the 8 groups of 16 partitions)."""
    arr = np.asarray(vals, np.int16).reshape(runs, run_len // 16, 16)
    arr = np.ascontiguousarray(arr.transpose(2, 0, 1).reshape(16, -1))
    return np.ascontiguousarray(np.tile(arr, (8, 1)))


def _host_prep(x, edge_index, edge_attr, emb_W, emb_b, edge_W, edge_b,
               U_W, U_b, V_W, V_b, A_W, A_b, B_W, B_b, E_W, E_b, ln_g, ln_b):
    N, IN_DIM = x.shape
    E = edge_index.shape[1]
    ED = edge_attr.shape[1]
    H = emb_W.shape[1]
    assert IN_DIM == H == P

    bpc = -(-N // (NCORES * P))          # blocks per core
    nblk = NCORES * bpc                  # total 128-node blocks
    npad = nblk * P
    nloc = bpc * P                       # node slots per core
    assert npad % NQ == 0
    qrows = npad // NQ

    src = np.ascontiguousarray(edge_index[0]).astype(np.int64)
    dst = np.ascontiguousarray(edge_index[1]).astype(np.int64)

    # --- degree-balanced node->block assignment (snake deal of sorted degrees)
    deg = np.bincount(dst, minlength=npad)
    order_nodes = np.argsort(-deg, kind="stable")
    assert npad % nblk == 0
    rounds = npad // nblk
    grid = order_nodes.reshape(rounds, nblk).copy()
    grid[1::2] = grid[1::2, ::-1]
    perm = np.empty(npad, dtype=np.int64)
    newids = (np.arange(nblk)[None, :] * P + np.arange(rounds)[:, None])
    perm[grid] = newids
    perm32 = perm.astype(np.int32)

    src_n = perm[src]
    dst_n = perm[dst]

    # --- sort edges by (dst block, src quarter); pad each run to capq
    q_of = src_n // qrows
    blk_of = dst_n >> 7
    key = blk_of * NQ + q_of
    eorder = np.argsort(key, kind="stable")
    key_s = key[eorder]
    src_s = src_n[eorder]
    dst_s = dst_n[eorder]
    q_s = q_of[eorder]
    ea_s = np.asarray(edge_attr, np.float32)[eorder]

    counts = np.bincount(key_s, minlength=nblk * NQ)
    capq = int(-(-counts.max() // P)) * P
    capb = NQ * capq                      # edge capacity per block
    nch = capb // P                       # chunks per block
    ccore = bpc * nch
    ecore = bpc * capb
    epad = nblk * capb

    run_start = np.zeros(nblk * NQ, dtype=np.int64)
    run_start[1:] = np.cumsum(counts)[:-1]
    rank = np.arange(E, dtype=np.int64) - run_start[key_s]
    pos = key_s * capq + rank

    srcq_p = np.zeros(epad, dtype=np.int16)      # pad 0 -> gathers row 0 (benign)
    dloc16_p = np.zeros(epad, dtype=np.int16)    # pad 0 -> onehot(0) (benign)
    dlocf_p = np.full(epad, 255, dtype=np.float32)  # pad 255 -> s4 all-zero row
    ea_p = np.zeros((epad, ED), dtype=np.float32)
    srcq_p[pos] = (src_s - q_s * qrows).astype(np.int16)
    dloc16_p[pos] = (dst_s & 127).astype(np.int16)
    dlocf_p[pos] = (dst_s & 127).astype(np.float32)
    ea_p[pos] = ea_s

    # --- fold weights (float64 host math, exact reassociation of reference)
    f8 = lambda a: np.asarray(a, np.float64)
    A2 = f8(emb_W) @ f8(A_W)
    U2 = f8(emb_W) @ f8(U_W)
    u2 = f8(emb_b) @ f8(U_W) + f8(U_b)
    V2 = f8(emb_W) @ f8(V_W)
    a2 = f8(emb_b) @ f8(A_W) + f8(A_b)
    v2 = f8(emb_b) @ f8(V_W) + f8(V_b)
    W2 = f8(edge_W) @ f8(E_W)
    b2 = f8(edge_b) @ f8(E_W) + f8(E_b) + a2 + v2

    bf = lambda a: np.ascontiguousarray(np.asarray(a, np.float32).astype(ml_dtypes.bfloat16))
    f32c = lambda a: np.ascontiguousarray(np.asarray(a, np.float32))

    consts = {
        "w2p": bf(np.concatenate([W2, b2[None, :]], axis=0)),       # [ED+1,128]
        "au2": bf(np.concatenate([A2, U2], axis=1)),                # [128,256]
        "v2w": f32c(V2),                                            # [128,128]
        "u2b": bf(np.tile(np.asarray(u2, np.float32)[None, :], (P, 1))),
        "embw": f32c(emb_W),
        "bw": f32c(B_W),
        "cb": f32c(np.tile((f8(emb_b) + f8(B_b))[None, :], (P, 1))),
        "iota": bf(np.tile(np.arange(P, dtype=np.float32)[None, :], (P, 1))),
        "identoh": bf(np.eye(P, dtype=np.float32)),
    }
    ln_affine = not (np.allclose(np.asarray(ln_g), 1.0) and np.allclose(np.asarray(ln_b), 0.0))
    if ln_affine:
        consts["gb"] = f32c(np.tile(np.asarray(ln_g, np.float32)[None, :], (P, 1)))
        consts["bb"] = f32c(np.tile(np.asarray(ln_b, np.float32)[None, :], (P, 1)))

    # --- x in permuted space: quarter tables (bf16 rows) + per-core f32 cols
    x_perm = np.zeros((npad, P), dtype=np.float32)
    x_perm[perm32[:N]] = np.asarray(x, np.float32)
    x_bf = np.ascontiguousarray(x_perm.astype(ml_dtypes.bfloat16))
    for q in range(NQ):
        consts[f"xq{q}"] = np.ascontiguousarray(x_bf[q * qrows:(q + 1) * qrows])

    # --- per-core arrays
    per_core = []
    for c in range(NCORES):
        s, e = c * ecore, (c + 1) * ecore
        eaT = np.concatenate([ea_p[s:e].T, np.ones((1, ecore), np.float32)], axis=0)
        per_core.append({
            "eat": np.ascontiguousarray(eaT.astype(ml_dtypes.bfloat16)),      # [ED+1, ecore]
            "dstloc": np.ascontiguousarray(dlocf_p[s:e].reshape(ccore, P).T),  # [128, ccore] f32
            "sidx": _wrap_idx16(srcq_p[s:e], bpc * NQ, capq),                  # [128, bpc*NQ*capq/16]
            "didx": _wrap_idx16(dloc16_p[s:e], bpc, capb),                     # [128, bpc*capb/16]
            "xtl": np.ascontiguousarray(x_perm[c * nloc:(c + 1) * nloc].T),    # [128, nloc] f32
        })

    meta = dict(N=N, E=E, ED=ED, npad=npad, nloc=nloc, bpc=bpc, qrows=qrows,
                capq=capq, capb=capb, nch=nch, ccore=ccore, ecore=ecore,
                perm32=perm32, ln_affine=ln_affine)
    return consts, per_core, meta


def _build_program(nc, tc, meta):
    ED = meta["ED"]
    nloc, bpc = meta["nloc"], meta["bpc"]
    qrows, capq, capb, nch, ccore = (
        meta["qrows"], meta["capq"], meta["capb"], meta["nch"], meta["ccore"])
    ln_affine = meta["ln_affine"]
    Alu = mybir.AluOpType
    Act = mybir.ActivationFunctionType
    cq16 = capq // 16
    cb16 = capb // 16

    def dram_in(name, shape, dt):
        return nc.dram_tensor(name, shape, dt, kind="ExternalInput").ap()

    xq_d = [dram_in(f"xq{q}", [qrows, P], BF16) for q in range(NQ)]
    w2p_d = dram_in("w2p", [ED + 1, P], BF16)
    au2_d = dram_in("au2", [P, 2 * P], BF16)
    v2_d = dram_in("v2w", [P, P], F32)
    u2b_d = dram_in("u2b", [P, P], BF16)
    embw_d = dram_in("embw", [P, P], F32)
    bw_d = dram_in("bw", [P, P], F32)
    cb_d = dram_in("cb", [P, P], F32)
    iota_d = dram_in("iota", [P, P], BF16)
    identoh_d = dram_in("identoh", [P, P], BF16)
    if ln_affine:
        gb_d = dram_in("gb", [P, P], F32)
        bb_d = dram_in("bb", [P, P], F32)
    eat_d = dram_in("eat", [ED + 1, meta["ecore"]], BF16)
    dstloc_d = dram_in("dstloc", [P, ccore], F32)
    sidx_d = dram_in("sidx", [P, bpc * NQ * cq16], I16)
    didx_d = dram_in("didx", [P, bpc * cb16], I16)
    xtl_d = dram_in("xtl", [P, nloc], F32)
    out_d = nc.dram_tensor("out", [nloc, P], F32, kind="ExternalOutput").ap()

    nc.gpsimd.load_library(library_config.mlp)

    ctx = ExitStack()
    with ctx:
        cpool = ctx.enter_context(tc.tile_pool(name="const", bufs=1))

        def load_const(src_ap, shape, dt, tag):
            t = cpool.tile(shape, dt, tag=tag)
            nc.sync.dma_start(out=t[:], in_=src_ap[:])
            return t

        w2p_sb = load_const(w2p_d, [ED + 1, P], BF16, "c_w2p")
        au2_sb = load_const(au2_d, [P, 2 * P], BF16, "c_au2")
        v2_sb = load_const(v2_d, [P, P], F32, "c_v2")
        u2b_sb = load_const(u2b_d, [P, P], BF16, "c_u2b")
        embw_sb = load_const(embw_d, [P, P], F32, "c_embw")
        bw_sb = load_const(bw_d, [P, P], F32, "c_bw")
        cb_sb = load_const(cb_d, [P, P], F32, "c_cb")
        iota_sb = load_const(iota_d, [P, P], BF16, "c_iota")
        identoh_sb = load_const(identoh_d, [P, P], BF16, "c_identoh")
        if ln_affine:
            gb_sb = load_const(gb_d, [P, P], F32, "c_gb")
            bb_sb = load_const(bb_d, [P, P], F32, "c_bb")
        dstloc_sb = load_const(dstloc_d, [P, ccore], F32, "c_dstloc")

        pb_bufs = int(os.environ.get("K_PB_BUFS", "3"))
        pc_bufs = int(os.environ.get("K_PC_BUFS", "8"))
        paup_bufs = int(os.environ.get("K_PAUP_BUFS", "3"))
        with tc.tile_pool(name="pb", bufs=pb_bufs) as pb, \
             tc.tile_pool(name="pc", bufs=pc_bufs) as pc, \
             tc.tile_pool(name="paup", bufs=paup_bufs, space="PSUM") as paup, \
             tc.tile_pool(name="p1p", bufs=2, space="PSUM") as p1p:
            for blk in range(bpc):
                # ---- block loads
                eat_t = pb.tile([ED + 1, capb], BF16, tag="eat")
                nc.sync.dma_start(out=eat_t[:],
                                  in_=eat_d[:, blk * capb:(blk + 1) * capb])
                sidx_t = pb.tile([P, NQ * cq16], I16, tag="sidx")
                nc.sync.dma_start(out=sidx_t[:],
                                  in_=sidx_d[:, blk * NQ * cq16:(blk + 1) * NQ * cq16])
                didx_t = pb.tile([P, cb16], I16, tag="didx")
                nc.sync.dma_start(out=didx_t[:],
                                  in_=didx_d[:, blk * cb16:(blk + 1) * cb16])
                xtl_t = pb.tile([P, P], F32, tag="xtl")
                nc.sync.dma_start(out=xtl_t[:], in_=xtl_d[:, blk * P:(blk + 1) * P])

                # ---- gathers: x[src]^T per quarter + onehot(dst) columns
                xg_t = pb.tile([P, capb], BF16, tag="xg")
                for q in range(NQ):
                    if os.environ.get("K_NOXG"):
                        break
                    nc.gpsimd.dma_gather(
                        out_ap=xg_t[:, q * capq:(q + 1) * capq]
                            .rearrange("p (o e) -> p o e", o=1),
                        in_ap=xq_d[q][:],
                        idxs_ap=sidx_t[:, q * cq16:(q + 1) * cq16],
                        num_idxs=capq,
                        num_idxs_reg=capq,
                        elem_size=P,
                        transpose=True,
                        single_packet=False,
                    )
                oh_t = pb.tile([P, capb], BF16, tag="oh")
                if os.environ.get("K_NOOH"):
                    nc.vector.tensor_copy(out=oh_t[:, 0:P], in_=identoh_sb[:])
                else:
                    nc.gpsimd.dma_gather(
                    out_ap=oh_t[:].rearrange("p (o e) -> p o e", o=1),
                    in_ap=identoh_sb[:],
                    idxs_ap=didx_t[:],
                    num_idxs=capb,
                    num_idxs_reg=capb,
                    elem_size=P,
                        transpose=True,
                        single_packet=False,
                        sbuf_tokens_per_rank=P,
                        sbuf_free_dim_per_rank=2 * P,
                    )

                # ---- per-block Vh table (f32 matmul) + u2 bias half
                vhb_ps = paup.tile([P, 2 * P], F32, tag="pau")
                nc.tensor.matmul(vhb_ps[:, 0:P], lhsT=xtl_t[:], rhs=v2_sb[:],
                                 start=True, stop=True)
                vhbu2_t = pb.tile([P, 2 * P], BF16, tag="vhbu2")
                nc.vector.tensor_copy(out=vhbu2_t[:, 0:P], in_=vhb_ps[:, 0:P])
                nc.vector.tensor_copy(out=vhbu2_t[:, P:2 * P], in_=u2b_sb[:])

                # ---- edge chunks, in groups of GRP: per-chunk gate matmuls into
                # a grouped PSUM tile, then one batched sigmoid (ACT) + one
                # batched msg multiply (DVE) per group via strided APs.
                GRP = 4
                assert nch % GRP == 0
                agg_delay = int(os.environ.get("K_AGG_DELAY", "2"))
                p1 = p1p.tile([P, P], F32, tag="p1")
                pend = []
                for g in range(nch // GRP):
                    pau = paup.tile([P, GRP * 2 * P], F32, tag="pau")
                    for j in range(GRP):
                        c = g * GRP + j
                        cs = slice(c * P, (c + 1) * P)
                        js = slice(j * 2 * P, (j + 1) * 2 * P)
                        jg = slice(j * 2 * P, j * 2 * P + P)
                        nc.tensor.matmul(pau[:, js], lhsT=xg_t[:, cs], rhs=au2_sb[:],
                                         start=True, stop=False)
                        nc.tensor.matmul(pau[:, jg], lhsT=eat_t[:, cs], rhs=w2p_sb[:],
                                         start=False, stop=False, skip_group_check=True)
                        nc.tensor.matmul(pau[:, js], lhsT=oh_t[:, cs], rhs=vhbu2_t[:],
                                         start=False, stop=True)
                    pau3 = pau[:].rearrange("p (c t e) -> p c t e", t=2, e=P)
                    gate4 = pc.tile([P, GRP * P], BF16, tag="gate")
                    nc.scalar.activation(
                        out=gate4[:].rearrange("p (c e) -> p c e", e=P),
                        in_=pau3[:, :, 0, :], func=Act.Sigmoid)
                    s44 = pc.tile([P, GRP * P], BF16, tag="s4")
                    for j in range(GRP):
                        gc = blk * nch + g * GRP + j
                        nc.vector.tensor_scalar(out=s44[:, j * P:(j + 1) * P],
                                                in0=iota_sb[:],
                                                scalar1=dstloc_sb[:, gc:gc + 1],
                                                scalar2=None, op0=Alu.is_equal)
                    msg4 = pc.tile([P, GRP * P], BF16, tag="msg")
                    nc.vector.tensor_tensor(
                        out=msg4[:].rearrange("p (c e) -> p c e", e=P),
                        in0=gate4[:].rearrange("p (c e) -> p c e", e=P),
                        in1=pau3[:, :, 1, :], op=Alu.mult)
                    pend.append((g, msg4, s44))
                    if len(pend) > agg_delay:
                        gg, m4, s4v = pend.pop(0)
                        for j in range(GRP):
                            c = gg * GRP + j
                            jp = slice(j * P, (j + 1) * P)
                            nc.tensor.matmul(p1[:], lhsT=m4[:, jp], rhs=s4v[:, jp],
                                             start=(c == 0), stop=(c == nch - 1))
                for gg, m4, s4v in pend:
                    for j in range(GRP):
                        c = gg * GRP + j
                        jp = slice(j * P, (j + 1) * P)
                        nc.tensor.matmul(p1[:], lhsT=m4[:, jp], rhs=s4v[:, jp],
                                         start=(c == 0), stop=(c == nch - 1))

                # ---- block tail: out = LN(h + aggr@B_W + cb)
                aggT = pb.tile([P, P], F32, tag="aggT")
                nc.vector.tensor_copy(out=aggT[:], in_=p1[:])
                p2 = paup.tile([P, 2 * P], F32, tag="pau")
                nc.tensor.matmul(p2[:, 0:P], lhsT=aggT[:], rhs=bw_sb[:], start=True, stop=False)
                nc.tensor.matmul(p2[:, 0:P], lhsT=xtl_t[:], rhs=embw_sb[:],
                                 start=False, stop=True)
                v = pb.tile([P, P], F32, tag="v")
                nc.vector.tensor_tensor(out=v[:], in0=p2[:, 0:P], in1=cb_sb[:], op=Alu.add)
                sum_t = pb.tile([P, 1], F32, tag="sum")
                nc.vector.tensor_reduce(out=sum_t[:], in_=v[:],
                                        axis=mybir.AxisListType.X, op=Alu.add)
                mu_t = pb.tile([P, 1], F32, tag="mu")
                nc.vector.tensor_scalar(out=mu_t[:], in0=sum_t[:], scalar1=1.0 / P,
                                        scalar2=None, op0=Alu.mult)
                vc = pb.tile([P, P], F32, tag="vc")
                nc.vector.tensor_scalar(out=vc[:], in0=v[:], scalar1=mu_t[:, :1],
                                        scalar2=None, op0=Alu.subtract)
                sq = pb.tile([P, P], F32, tag="sq")
                nc.scalar.activation(out=sq[:], in_=vc[:], func=Act.Square)
                var_t = pb.tile([P, 1], F32, tag="var")
                nc.vector.tensor_reduce(out=var_t[:], in_=sq[:],
                                        axis=mybir.AxisListType.X, op=Alu.add)
                # rstd = 1/sqrt(var+eps) on DVE (bit-trick + 2 Newton steps)
                # so ACT stays sigmoid-set only (no act-table reloads).
                h_t = pb.tile([P, 1], F32, tag="h")
                nc.vector.tensor_scalar(out=h_t[:], in0=var_t[:], scalar1=0.5 / P,
                                        scalar2=0.5e-5, op0=Alu.mult, op1=Alu.add)
                y0i = pb.tile([P, 1], mybir.dt.int32, tag="y0i")
                nc.vector.tensor_scalar(out=y0i[:], in0=h_t[:].bitcast(mybir.dt.int32),
                                        scalar1=1, scalar2=-1,
                                        op0=Alu.logical_shift_right, op1=Alu.bitwise_xor)
                # y0i = ~(ih>>1); add (magic'+1) => magic' - (ih>>1), magic'=0x5EF759DF
                nc.vector.tensor_scalar(out=y0i[:], in0=y0i[:], scalar1=0x5EF759E0,
                                        scalar2=None, op0=Alu.add)
                y0 = y0i[:].bitcast(F32)
                t4 = pb.tile([P, 1], F32, tag="t4")
                rstd = pb.tile([P, 1], F32, tag="rstd")
                nc.vector.tensor_tensor(out=t4[:], in0=y0, in1=y0, op=Alu.mult)
                nc.vector.tensor_tensor(out=t4[:], in0=t4[:], in1=h_t[:], op=Alu.mult)
                nc.vector.tensor_scalar(out=t4[:], in0=t4[:], scalar1=-1.0,
                                        scalar2=1.5, op0=Alu.mult, op1=Alu.add)
                nc.vector.tensor_tensor(out=rstd[:], in0=y0, in1=t4[:], op=Alu.mult)
                nc.vector.tensor_tensor(out=t4[:], in0=rstd[:], in1=rstd[:], op=Alu.mult)
                nc.vector.tensor_tensor(out=t4[:], in0=t4[:], in1=h_t[:], op=Alu.mult)
                nc.vector.tensor_scalar(out=t4[:], in0=t4[:], scalar1=-1.0,
                                        scalar2=1.5, op0=Alu.mult, op1=Alu.add)
                nc.vector.tensor_tensor(out=rstd[:], in0=rstd[:], in1=t4[:], op=Alu.mult)
                outb = pb.tile([P, P], F32, tag="outb")
                nc.vector.tensor_scalar(out=outb[:], in0=vc[:], scalar1=rstd[:, :1],
                                        scalar2=None, op0=Alu.mult)
                if ln_affine:
                    nc.vector.tensor_tensor(out=outb[:], in0=outb[:], in1=gb_sb[:], op=Alu.mult)
                    nc.vector.tensor_tensor(out=outb[:], in0=outb[:], in1=bb_sb[:], op=Alu.add)
                nc.sync.dma_start(out=out_d[blk * P:(blk + 1) * P, :], in_=outb[:])


def _build(inputs):
    consts, per_core, meta = _host_prep(**inputs)
    nc = bacc.Bacc("TRN2", target_bir_lowering=False, debug=False,
                   num_devices=NCORES)
    with tile.TileContext(nc) as tc:
        _build_program(nc, tc, meta)
    nc.compile()
    in_maps = [{**consts, **per_core[c]} for c in range(NCORES)]
    return dict(nc=nc, in_maps=in_maps, meta=meta)


def _exec(ctx, trace=False):
    global LAST_RESULTS
    res = bass_utils.run_bass_kernel_spmd(
        ctx["nc"], ctx["in_maps"], core_ids=list(range(NCORES)), trace=trace)
    LAST_RESULTS = res
    meta = ctx["meta"]
    big = np.concatenate([res.results[c]["out"] for c in range(NCORES)], axis=0)
    out = big[meta["perm32"][:meta["N"]]]
    return np.ascontiguousarray(out, dtype=np.float32)


def _timeit(ctx, iters=5):
    """Steady-state per-call wall time with device-resident inputs (upper
    bound on HW exec: includes dispatch/axon overhead but no H2D)."""
    import time
    import jax
    from jax.experimental.shard_map import shard_map
    from jax.sharding import Mesh, PartitionSpec, NamedSharding
    from concourse import bass2jax as b2j
    from concourse import mybir as _mb

    nc = ctx["nc"]
    in_maps = ctx["in_maps"]
    in_names, out_names, out_avals, zero_outs = [], [], [], []
    part_name = nc.partition_id_tensor.name if nc.partition_id_tensor else None
    for alloc in nc.m.functions[0].allocations:
        if not isinstance(alloc, _mb.MemoryLocationSet):
            continue
        name = alloc.memorylocations[0].name
        if alloc.kind == "ExternalInput":
            if name != part_name:
                in_names.append(name)
        elif alloc.kind == "ExternalOutput":
            out_names.append(name)
            shape = tuple(alloc.tensor_shape)
            dtype = _mb.dt.np(alloc.dtype)
            out_avals.append(jax.core.ShapedArray(shape, dtype))
            zero_outs.append(np.zeros(shape, dtype))
    n_params = len(in_names)
    all_names = in_names + out_names
    if part_name is not None:
        all_names = all_names + [part_name]

    def _body(*args):
        operands = list(args)
        if part_name is not None:
            operands.append(b2j.partition_id_tensor())
        outs = b2j._bass_exec_p.bind(
            *operands, out_avals=tuple(out_avals), in_names=tuple(all_names),
            out_names=tuple(out_names), lowering_input_output_aliases=(),
            sim_require_finite=True, sim_require_nnan=True, nc=nc)
        return tuple(outs)

    devices = jax.devices()[:NCORES]
    mesh = Mesh(np.asarray(devices), ("core",))
    spec = PartitionSpec("core")
    n_outs = len(out_names)
    fn = jax.jit(shard_map(_body, mesh=mesh,
                           in_specs=(spec,) * (n_params + n_outs),
                           out_specs=(spec,) * n_outs, check_rep=False))
    sharding = NamedSharding(mesh, spec)
    dev_in = [jax.device_put(
        np.concatenate([np.asarray(in_maps[c][nm]) for c in range(NCORES)], axis=0),
        sharding) for nm in in_names]
    dev_zero = [jax.device_put(
        np.zeros((NCORES * z.shape[0], *z.shape[1:]), z.dtype), sharding)
        for z in zero_outs]
    times = []
    out = None
    for _ in range(iters):
        t0 = time.perf_counter()
        out = fn(*dev_in, *dev_zero)
        jax.block_until_ready(out)
        times.append(time.perf_counter() - t0)
    return times, out


def kernel(**inputs) -> np.ndarray:
    return _exec(_build(inputs))


# revision 18
# speedup vs baseline: 1.3824x; 1.3824x over previous
"""GatedGCN LocalEncoder kernel for 8x Trainium2 NeuronCores (Bass/Tile).

Strategy: destination-sorted edge sharding with dma_gather-based source
gathers. Nodes are relabeled into degree-balanced 128-node blocks (784
blocks, 98 per core). All edges with dst in a block form one contiguous
run, sub-sorted by source quarter (4 tables of 25088 rows so gather
indices fit int16). Per block:
  - 4 transposed dma_gathers fetch x[src]^T (bf16) from the quarter tables
  - 1 SBUF-source dma_gather fetches onehot(dst_local) columns from a
    resident identity tile (used to broadcast the per-block Vh and the u2
    bias into the gate PSUM with a single matmul)
  - per 128-edge chunk: 3 gate matmuls (A|U projection from gathered x^T,
    edge-feature term, Vh/u2 one-hot term), sigmoid, msg multiply, and a
    one-hot segment-sum matmul accumulated in PSUM
  - block tail: residual + LayerNorm as two f32 matmuls + DVE ops.
No cross-core communication; each core owns 98 blocks of destinations.
"""

import os
import sys
from contextlib import ExitStack

for _p in ("/opt/trn_rl_repo", os.path.expanduser("~/.axon_site/_ro/trn_rl_repo")):
    if os.path.isdir(_p) and _p not in sys.path:
        sys.path.insert(0, _p)

import numpy as np
import ml_dtypes

import concourse.bass as bass
import concourse.mybir as mybir
import concourse.tile as tile
from concourse import bacc
from concourse import bass_utils
from concourse import library_config

BF16 = mybir.dt.bfloat16
F32 = mybir.dt.float32
I16 = mybir.dt.int16
P = 128
NCORES = 8
NQ = 4  # source-quarter tables (int16 gather index limit)

LAST_RESULTS = None  # test harness introspection


def _wrap_idx16(vals, runs, run_len):
    """vals [runs*run_len] int -> [128, runs*run_len//16] int16 in dma_gather's
    wrapped layout (idx i of a run -> partition i%16, col i//16; replicated
    across the 8 groups of 16 partitions)."""
    arr = np.asarray(vals, np.int16).reshape(runs, run_len // 16, 16)
    arr = np.ascontiguousarray(arr.transpose(2, 0, 1).reshape(16, -1))
    return np.ascontiguousarray(np.tile(arr, (8, 1)))


def _host_prep(x, edge_index, edge_attr, emb_W, emb_b, edge_W, edge_b,
               U_W, U_b, V_W, V_b, A_W, A_b, B_W, B_b, E_W, E_b, ln_g, ln_b):
    N, IN_DIM = x.shape
    E = edge_index.shape[1]
    ED = edge_attr.shape[1]
    H = emb_W.shape[1]
    assert IN_DIM == H == P

    bpc = -(-N // (NCORES * P))          # blocks per core
    nblk = NCORES * bpc                  # total 128-node blocks
    npad = nblk * P
    nloc = bpc * P                       # node slots per core
    assert npad % NQ == 0
    qrows = npad // NQ

    src = np.ascontiguousarray(edge_index[0]).astype(np.int64)
    dst = np.ascontiguousarray(edge_index[1]).astype(np.int64)

    # --- degree-balanced node->block assignment (snake deal of sorted degrees)
    deg = np.bincount(dst, minlength=npad)
    order_nodes = np.argsort(-deg, kind="stable")
    assert npad % nblk == 0
    rounds = npad // nblk
    grid = order_nodes.reshape(rounds, nblk).copy()
    grid[1::2] = grid[1::2, ::-1]
    perm = np.empty(npad, dtype=np.int64)
    newids = (np.arange(nblk)[None, :] * P + np.arange(rounds)[:, None])
    perm[grid] = newids
    perm32 = perm.astype(np.int32)

    src_n = perm[src]
    dst_n = perm[dst]

    # --- sort edges by (dst block, src quarter); pad each run to capq
    q_of = src_n // qrows
    blk_of = dst_n >> 7
    key = blk_of * NQ + q_of
    eorder = np.argsort(key, kind="stable")
    key_s = key[eorder]
    src_s = src_n[eorder]
    dst_s = dst_n[eorder]
    q_s = q_of[eorder]
    ea_s = np.asarray(edge_attr, np.float32)[eorder]

    counts = np.bincount(key_s, minlength=nblk * NQ)
    capq = int(-(-counts.max() // P)) * P
    capb = NQ * capq                      # edge capacity per block
    nch = capb // P                       # chunks per block
    ccore = bpc * nch
    ecore = bpc * capb
    epad = nblk * capb

    run_start = np.zeros(nblk * NQ, dtype=np.int64)
    run_start[1:] = np.cumsum(counts)[:-1]
    rank = np.arange(E, dtype=np.int64) - run_start[key_s]
    pos = key_s * capq + rank

    srcq_p = np.zeros(epad, dtype=np.int16)      # pad 0 -> gathers row 0 (benign)
    dloc16_p = np.zeros(epad, dtype=np.int16)    # pad 0 -> onehot(0) (benign)
    dlocf_p = np.full(epad, 255, dtype=np.float32)  # pad 255 -> s4 all-zero row
    ea_p = np.zeros((epad, ED), dtype=np.float32)
    srcq_p[pos] = (src_s - q_s * qrows).astype(np.int16)
    dloc16_p[pos] = (dst_s & 127).astype(np.int16)
    dlocf_p[pos] = (dst_s & 127).astype(np.float32)
    ea_p[pos] = ea_s

    # --- fold weights (float64 host math, exact reassociation of reference)
    f8 = lambda a: np.asarray(a, np.float64)
    A2 = f8(emb_W) @ f8(A_W)
    U2 = f8(emb_W) @ f8(U_W)
    u2 = f8(emb_b) @ f8(U_W) + f8(U_b)
    V2 = f8(emb_W) @ f8(V_W)
    a2 = f8(emb_b) @ f8(A_W) + f8(A_b)
    v2 = f8(emb_b) @ f8(V_W) + f8(V_b)
    W2 = f8(edge_W) @ f8(E_W)
    b2 = f8(edge_b) @ f8(E_W) + f8(E_b) + a2 + v2

    bf = lambda a: np.ascontiguousarray(np.asarray(a, np.float32).astype(ml_dtypes.bfloat16))
    f32c = lambda a: np.ascontiguousarray(np.asarray(a, np.float32))

    consts = {
        "w2p": bf(np.concatenate([W2, b2[None, :]], axis=0)),       # [ED+1,128]
        "au2": bf(np.concatenate([A2, U2], axis=1)),                # [128,256]
        "v2w": f32c(V2),                                            # [128,128]
        "u2b": bf(np.tile(np.asarray(u2, np.float32)[None, :], (P, 1))),
        "embw": f32c(emb_W),
        "bw": f32c(B_W),
        "cb": f32c(np.tile((f8(emb_b) + f8(B_b))[None, :], (P, 1))),
        "iota": bf(np.tile(np.arange(P, dtype=np.float32)[None, :], (P, 1))),
        "identoh": bf(np.eye(P, dtype=np.float32)),
    }
    ln_affine = not (np.allclose(np.asarray(ln_g), 1.0) and np.allclose(np.asarray(ln_b), 0.0))
    if ln_affine:
        consts["gb"] = f32c(np.tile(np.asarray(ln_g, np.float32)[None, :], (P, 1)))
        consts["bb"] = f32c(np.tile(np.asarray(ln_b, np.float32)[None, :], (P, 1)))

    # --- x in permuted space: quarter tables (bf16 rows) + per-core f32 cols
    x_perm = np.zeros((npad, P), dtype=np.float32)
    x_perm[perm32[:N]] = np.asarray(x, np.float32)
    x_bf = np.ascontiguousarray(x_perm.astype(ml_dtypes.bfloat16))
    for q in range(NQ):
        consts[f"xq{q}"] = np.ascontiguousarray(x_bf[q * qrows:(q + 1) * qrows])

    # --- per-core arrays
    per_core = []
    for c in range(NCORES):
        s, e = c * ecore, (c + 1) * ecore
        eaT = np.concatenate([ea_p[s:e].T, np.ones((1, ecore), np.float32)], axis=0)
        per_core.append({
            "eat": np.ascontiguousarray(eaT.astype(ml_dtypes.bfloat16)),      # [ED+1, ecore]
            "dstloc": np.ascontiguousarray(dlocf_p[s:e].reshape(ccore, P).T),  # [128, ccore] f32
            "sidx": _wrap_idx16(srcq_p[s:e], bpc * NQ, capq),                  # [128, bpc*NQ*capq/16]
            "didx": _wrap_idx16(dloc16_p[s:e], bpc, capb),                     # [128, bpc*capb/16]
            "xtl": np.ascontiguousarray(x_perm[c * nloc:(c + 1) * nloc].T),    # [128, nloc] f32
        })

    meta = dict(N=N, E=E, ED=ED, npad=npad, nloc=nloc, bpc=bpc, qrows=qrows,
                capq=capq, capb=capb, nch=nch, ccore=ccore, ecore=ecore,
                perm32=perm32, ln_affine=ln_affine)
    return consts, per_core, meta


def _build_program(nc, tc, meta):
    ED = meta["ED"]
    nloc, bpc = meta["nloc"], meta["bpc"]
    qrows, capq, capb, nch, ccore = (
        meta["qrows"], meta["capq"], meta["capb"], meta["nch"], meta["ccore"])
    ln_affine = meta["ln_affine"]
    Alu = mybir.AluOpType
    Act = mybir.ActivationFunctionType
    cq16 = capq // 16
    cb16 = capb // 16

    def dram_in(name, shape, dt):
        return nc.dram_tensor(name, shape, dt, kind="ExternalInput").ap()

    xq_d = [dram_in(f"xq{q}", [qrows, P], BF16) for q in range(NQ)]
    w2p_d = dram_in("w2p", [ED + 1, P], BF16)
    au2_d = dram_in("au2", [P, 2 * P], BF16)
    v2_d = dram_in("v2w", [P, P], F32)
    u2b_d = dram_in("u2b", [P, P], BF16)
    embw_d = dram_in("embw", [P, P], F32)
    bw_d = dram_in("bw", [P, P], F32)
    cb_d = dram_in("cb", [P, P], F32)
    iota_d = dram_in("iota", [P, P], BF16)
    identoh_d = dram_in("identoh", [P, P], BF16)
    if ln_affine:
        gb_d = dram_in("gb", [P, P], F32)
        bb_d = dram_in("bb", [P, P], F32)
    eat_d = dram_in("eat", [ED + 1, meta["ecore"]], BF16)
    dstloc_d = dram_in("dstloc", [P, ccore], F32)
    sidx_d = dram_in("sidx", [P, bpc * NQ * cq16], I16)
    didx_d = dram_in("didx", [P, bpc * cb16], I16)
    xtl_d = dram_in("xtl", [P, nloc], F32)
    out_d = nc.dram_tensor("out", [nloc, P], F32, kind="ExternalOutput").ap()

    nc.gpsimd.load_library(library_config.mlp)

    ctx = ExitStack()
    with ctx:
        cpool = ctx.enter_context(tc.tile_pool(name="const", bufs=1))

        def load_const(src_ap, shape, dt, tag):
            t = cpool.tile(shape, dt, tag=tag)
            nc.sync.dma_start(out=t[:], in_=src_ap[:])
            return t

        w2p_sb = load_const(w2p_d, [ED + 1, P], BF16, "c_w2p")
        au2_sb = load_const(au2_d, [P, 2 * P], BF16, "c_au2")
        v2_sb = load_const(v2_d, [P, P], F32, "c_v2")
        u2b_sb = load_const(u2b_d, [P, P], BF16, "c_u2b")
        embw_sb = load_const(embw_d, [P, P], F32, "c_embw")
        bw_sb = load_const(bw_d, [P, P], F32, "c_bw")
        cb_sb = load_const(cb_d, [P, P], F32, "c_cb")
        iota_sb = load_const(iota_d, [P, P], BF16, "c_iota")
        identoh_sb = load_const(identoh_d, [P, P], BF16, "c_identoh")
        if ln_affine:
            gb_sb = load_const(gb_d, [P, P], F32, "c_gb")
            bb_sb = load_const(bb_d, [P, P], F32, "c_bb")
        dstloc_sb = load_const(dstloc_d, [P, ccore], F32, "c_dstloc")

        pb_bufs = int(os.environ.get("K_PB_BUFS", "3"))
        pc_bufs = int(os.environ.get("K_PC_BUFS", "8"))
        paup_bufs = int(os.environ.get("K_PAUP_BUFS", "3"))
        with tc.tile_pool(name="pb", bufs=pb_bufs) as pb, \
             tc.tile_pool(name="pc", bufs=pc_bufs) as pc, \
             tc.tile_pool(name="paup", bufs=paup_bufs, space="PSUM") as paup, \
             tc.tile_pool(name="p1p", bufs=2, space="PSUM") as p1p:
            for blk in range(bpc):
                # ---- block loads
                eat_t = pb.tile([ED + 1, capb], BF16, tag="eat")
                nc.sync.dma_start(out=eat_t[:],
                                  in_=eat_d[:, blk * capb:(blk + 1) * capb])
                sidx_t = pb.tile([P, NQ * cq16], I16, tag="sidx")
                nc.sync.dma_start(out=sidx_t[:],
                                  in_=sidx_d[:, blk * NQ * cq16:(blk + 1) * NQ * cq16])
                didx_t = pb.tile([P, cb16], I16, tag="didx")
                nc.sync.dma_start(out=didx_t[:],
                                  in_=didx_d[:, blk * cb16:(blk + 1) * cb16])
                xtl_t = pb.tile([P, P], F32, tag="xtl")
                nc.sync.dma_start(out=xtl_t[:], in_=xtl_d[:, blk * P:(blk + 1) * P])

                # ---- gathers: x[src]^T per quarter + onehot(dst) columns
                xg_t = pb.tile([P, capb], BF16, tag="xg")
                for q in range(NQ):
                    if os.environ.get("K_NOXG"):
                        break
                    nc.gpsimd.dma_gather(
                        out_ap=xg_t[:, q * capq:(q + 1) * capq]
                            .rearrange("p (o e) -> p o e", o=1),
                        in_ap=xq_d[q][:],
                        idxs_ap=sidx_t[:, q * cq16:(q + 1) * cq16],
                        num_idxs=capq,
                        num_idxs_reg=capq,
                        elem_size=P,
                        transpose=True,
                        single_packet=False,
                    )
                oh_t = pb.tile([P, capb], BF16, tag="oh")
                if os.environ.get("K_NOOH"):
                    nc.vector.tensor_copy(out=oh_t[:, 0:P], in_=identoh_sb[:])
                else:
                    nc.gpsimd.dma_gather(
                    out_ap=oh_t[:].rearrange("p (o e) -> p o e", o=1),
                    in_ap=identoh_sb[:],
                    idxs_ap=didx_t[:],
                    num_idxs=capb,
                    num_idxs_reg=capb,
                    elem_size=P,
                        transpose=True,
                        single_packet=False,
                        sbuf_tokens_per_rank=P,
                        sbuf_free_dim_per_rank=2 * P,
                    )

                # ---- per-block Vh table (f32 matmul) + u2 bias half
                vhb_ps = paup.tile([P, 2 * P], F32, tag="pau")
                nc.tensor.matmul(vhb_ps[:, 0:P], lhsT=xtl_t[:], rhs=v2_sb[:],
                                 start=True, stop=True)
                vhbu2_t = pb.tile([P, 2 * P], BF16, tag="vhbu2")
                nc.vector.tensor_copy(out=vhbu2_t[:, 0:P], in_=vhb_ps[:, 0:P])
                nc.vector.tensor_copy(out=vhbu2_t[:, P:2 * P], in_=u2b_sb[:])

                # ---- edge chunks, in groups of GRP: per-chunk gate matmuls into
                # a grouped PSUM tile, then one batched sigmoid (ACT) + one
                # batched msg multiply (DVE) per group via strided APs.
                GRP = 4
                assert nch % GRP == 0
                agg_delay = int(os.environ.get("K_AGG_DELAY", "3"))
                p1 = p1p.tile([P, P], F32, tag="p1")
                pend = []
                for g in range(nch // GRP):
                    pau = paup.tile([P, GRP * 2 * P], F32, tag="pau")
                    for j in range(GRP):
                        c = g * GRP + j
                        cs = slice(c * P, (c + 1) * P)
                        js = slice(j * 2 * P, (j + 1) * 2 * P)
                        jg = slice(j * 2 * P, j * 2 * P + P)
                        nc.tensor.matmul(pau[:, js], lhsT=xg_t[:, cs], rhs=au2_sb[:],
                                         start=True, stop=False)
                        nc.tensor.matmul(pau[:, jg], lhsT=eat_t[:, cs], rhs=w2p_sb[:],
                                         start=False, stop=False, skip_group_check=True)
                        nc.tensor.matmul(pau[:, js], lhsT=oh_t[:, cs], rhs=vhbu2_t[:],
                                         start=False, stop=True)
                    pau3 = pau[:].rearrange("p (c t e) -> p c t e", t=2, e=P)
                    gate4 = pc.tile([P, GRP * P], BF16, tag="gate")
                    nc.scalar.activation(
                        out=gate4[:].rearrange("p (c e) -> p c e", e=P),
                        in_=pau3[:, :, 0, :], func=Act.Sigmoid)
                    s44 = pc.tile([P, GRP * P], BF16, tag="s4")
                    for j in range(GRP):
                        gc = blk * nch + g * GRP + j
                        nc.vector.tensor_scalar(out=s44[:, j * P:(j + 1) * P],
                                                in0=iota_sb[:],
                                                scalar1=dstloc_sb[:, gc:gc + 1],
                                                scalar2=None, op0=Alu.is_equal)
                    msg4 = pc.tile([P, GRP * P], BF16, tag="msg")
                    nc.vector.tensor_tensor(
                        out=msg4[:].rearrange("p (c e) -> p c e", e=P),
                        in0=gate4[:].rearrange("p (c e) -> p c e", e=P),
                        in1=pau3[:, :, 1, :], op=Alu.mult)
                    pend.append((g, msg4, s44))
                    if len(pend) > agg_delay:
                        gg, m4, s4v = pend.pop(0)
                        for j in range(GRP):
                            c = gg * GRP + j
                            jp = slice(j * P, (j + 1) * P)
                            nc.tensor.matmul(p1[:], lhsT=m4[:, jp], rhs=s4v[:, jp],
                                             start=(c == 0), stop=(c == nch - 1))
                for gg, m4, s4v in pend:
                    for j in range(GRP):
                        c = gg * GRP + j
                        jp = slice(j * P, (j + 1) * P)
                        nc.tensor.matmul(p1[:], lhsT=m4[:, jp], rhs=s4v[:, jp],
                                         start=(c == 0), stop=(c == nch - 1))

                # ---- block tail: out = LN(h + aggr@B_W + cb)
                aggT = pb.tile([P, P], F32, tag="aggT")
                nc.vector.tensor_copy(out=aggT[:], in_=p1[:])
                p2 = paup.tile([P, 2 * P], F32, tag="pau")
                nc.tensor.matmul(p2[:, 0:P], lhsT=aggT[:], rhs=bw_sb[:], start=True, stop=False)
                nc.tensor.matmul(p2[:, 0:P], lhsT=xtl_t[:], rhs=embw_sb[:],
                                 start=False, stop=True)
                v = pb.tile([P, P], F32, tag="v")
                nc.vector.tensor_tensor(out=v[:], in0=p2[:, 0:P], in1=cb_sb[:], op=Alu.add)
                sum_t = pb.tile([P, 1], F32, tag="sum")
                nc.vector.tensor_reduce(out=sum_t[:], in_=v[:],
                                        axis=mybir.AxisListType.X, op=Alu.add)
                mu_t = pb.tile([P, 1], F32, tag="mu")
                nc.vector.tensor_scalar(out=mu_t[:], in0=sum_t[:], scalar1=1.0 / P,
                                        scalar2=None, op0=Alu.mult)
                vc = pb.tile([P, P], F32, tag="vc")
                nc.vector.tensor_scalar(out=vc[:], in0=v[:], scalar1=mu_t[:, :1],
                                        scalar2=None, op0=Alu.subtract)
                sq = pb.tile([P, P], F32, tag="sq")
                nc.scalar.activation(out=sq[:], in_=vc[:], func=Act.Square)
                var_t = pb.tile([P, 1], F32, tag="var")
                nc.vector.tensor_reduce(out=var_t[:], in_=sq[:],
                                        axis=mybir.AxisListType.X, op=Alu.add)
                # rstd = 1/sqrt(var+eps) on DVE (bit-trick + 2 Newton steps)
                # so ACT stays sigmoid-set only (no act-table reloads).
                h_t = pb.tile([P, 1], F32, tag="h")
                nc.vector.tensor_scalar(out=h_t[:], in0=var_t[:], scalar1=0.5 / P,
                                        scalar2=0.5e-5, op0=Alu.mult, op1=Alu.add)
                y0i = pb.tile([P, 1], mybir.dt.int32, tag="y0i")
                nc.vector.tensor_scalar(out=y0i[:], in0=h_t[:].bitcast(mybir.dt.int32),
                                        scalar1=1, scalar2=-1,
                                        op0=Alu.logical_shift_right, op1=Alu.bitwise_xor)
                # y0i = ~(ih>>1); add (magic'+1) => magic' - (ih>>1), magic'=0x5EF759DF
                nc.vector.tensor_scalar(out=y0i[:], in0=y0i[:], scalar1=0x5EF759E0,
                                        scalar2=None, op0=Alu.add)
                y0 = y0i[:].bitcast(F32)
                t4 = pb.tile([P, 1], F32, tag="t4")
                rstd = pb.tile([P, 1], F32, tag="rstd")
                nc.vector.tensor_tensor(out=t4[:], in0=y0, in1=y0, op=Alu.mult)
                nc.vector.tensor_tensor(out=t4[:], in0=t4[:], in1=h_t[:], op=Alu.mult)
                nc.vector.tensor_scalar(out=t4[:], in0=t4[:], scalar1=-1.0,
                                        scalar2=1.5, op0=Alu.mult, op1=Alu.add)
                nc.vector.tensor_tensor(out=rstd[:], in0=y0, in1=t4[:], op=Alu.mult)
                nc.vector.tensor_tensor(out=t4[:], in0=rstd[:], in1=rstd[:], op=Alu.mult)
                nc.vector.tensor_tensor(out=t4[:], in0=t4[:], in1=h_t[:], op=Alu.mult)
                nc.vector.tensor_scalar(out=t4[:], in0=t4[:], scalar1=-1.0,
                                        scalar2=1.5, op0=Alu.mult, op1=Alu.add)
                nc.vector.tensor_tensor(out=rstd[:], in0=rstd[:], in1=t4[:], op=Alu.mult)
                outb = pb.tile([P, P], F32, tag="outb")
                nc.vector.tensor_scalar(out=outb[:], in0=vc[:], scalar1=rstd[:, :1],
                                        scalar2=None, op0=Alu.mult)
                if ln_affine:
                    nc.vector.tensor_tensor(out=outb[:], in0=outb[:], in1=gb_sb[:], op=Alu.mult)
                    nc.vector.tensor_tensor(out=outb[:], in0=outb[:], in1=bb_sb[:], op=Alu.add)
                nc.sync.dma_start(out=out_d[blk * P:(blk + 1) * P, :], in_=outb[:])


def _build(inputs):
    consts, per_core, meta = _host_prep(**inputs)
    nc = bacc.Bacc("TRN2", target_bir_lowering=False, debug=False,
                   num_devices=NCORES)
    with tile.TileContext(nc) as tc:
        _build_program(nc, tc, meta)
    nc.compile()
    in_maps = [{**consts, **per_core[c]} for c in range(NCORES)]
    return dict(nc=nc, in_maps=in_maps, meta=meta)


def _exec(ctx, trace=False):
    global LAST_RESULTS
    res = bass_utils.run_bass_kernel_spmd(
        ctx["nc"], ctx["in_maps"], core_ids=list(range(NCORES)), trace=trace)
    LAST_RESULTS = res
    meta = ctx["meta"]
    big = np.concatenate([res.results[c]["out"] for c in range(NCORES)], axis=0)
    out = big[meta["perm32"][:meta["N"]]]
    return np.ascontiguousarray(out, dtype=np.float32)


def _timeit(ctx, iters=5):
    """Steady-state per-call wall time with device-resident inputs (upper
    bound on HW exec: includes dispatch/axon overhead but no H2D)."""
    import time
    import jax
    from jax.experimental.shard_map import shard_map
    from jax.sharding import Mesh, PartitionSpec, NamedSharding
    from concourse import bass2jax as b2j
    from concourse import mybir as _mb

    nc = ctx["nc"]
    in_maps = ctx["in_maps"]
    in_names, out_names, out_avals, zero_outs = [], [], [], []
    part_name = nc.partition_id_tensor.name if nc.partition_id_tensor else None
    for alloc in nc.m.functions[0].allocations:
        if not isinstance(alloc, _mb.MemoryLocationSet):
            continue
        name = alloc.memorylocations[0].name
        if alloc.kind == "ExternalInput":
            if name != part_name:
                in_names.append(name)
        elif alloc.kind == "ExternalOutput":
            out_names.append(name)
            shape = tuple(alloc.tensor_shape)
            dtype = _mb.dt.np(alloc.dtype)
            out_avals.append(jax.core.ShapedArray(shape, dtype))
            zero_outs.append(np.zeros(shape, dtype))
    n_params = len(in_names)
    all_names = in_names + out_names
    if part_name is not None:
        all_names = all_names + [part_name]

    def _body(*args):
        operands = list(args)
        if part_name is not None:
            operands.append(b2j.partition_id_tensor())
        outs = b2j._bass_exec_p.bind(
            *operands, out_avals=tuple(out_avals), in_names=tuple(all_names),
            out_names=tuple(out_names), lowering_input_output_aliases=(),
            sim_require_finite=True, sim_require_nnan=True, nc=nc)
        return tuple(outs)

    devices = jax.devices()[:NCORES]
    mesh = Mesh(np.asarray(devices), ("core",))
    spec = PartitionSpec("core")
    n_outs = len(out_names)
    fn = jax.jit(shard_map(_body, mesh=mesh,
                           in_specs=(spec,) * (n_params + n_outs),
                           out_specs=(spec,) * n_outs, check_rep=False))
    sharding = NamedSharding(mesh, spec)
    dev_in = [jax.device_put(
        np.concatenate([np.asarray(in_maps[c][nm]) for c in range(NCORES)], axis=0),
        sharding) for nm in in_names]
    dev_zero = [jax.device_put(
        np.zeros((NCORES * z.shape[0], *z.shape[1:]), z.dtype), sharding)
        for z in zero_outs]
    times = []
    out = None
    for _ in range(iters):
        t0 = time.perf_counter()
        out = fn(*dev_in, *dev_zero)
        jax.block_until_ready(out)
        times.append(time.perf_counter() - t0)
    return times, out


def kernel(**inputs) -> np.ndarray:
    return _exec(_build(inputs))
